# revision 1
# baseline (speedup 1.0000x reference)
"""Capsule-routing kernel v2 — batch-merged, parity-packed (see kernel.py docstring).

Per core: 8 batches in 2 groups of 4, processed in lockstep so that matmuls
merge across batches and vector/scalar ops run at full [128, *] width.

Index conventions (per group of GB=4 batches):
  capsule n = 2*kc + parity   (kc in [0,16), parity in {0,1})
  slot(b, kc) = b*16 + kc     in [0, 64)
  p'(b, n)  = parity*64 + slot  -> o-all / oT / z column order  (parity-major)
  b/c layout: [128 (parity*64+G), 64 slot, 32 g]
Row-packed MM pairs: even capsules use partitions 0:64, odd 64:128, running
concurrently in distinct PE row groups (bf16 only — fp32 base-64 is broken).
"""

import numpy as np

B, IN_CAPS, IN_DIM = 64, 2048, 64
NUM, DIM = 32, 64
N_CORES = 8
BPC = B // N_CORES  # 8 batches per core
GB = 4              # batches per merged group
NG = BPC // GB      # 2 groups
EPS = 1e-7

_CACHE = {}


def _build_nc(bpc=BPC):
    import concourse.bacc as bacc
    import concourse.tile as tile
    from concourse import mybir

    f32 = mybir.dt.float32
    bf16 = mybir.dt.bfloat16
    Act = mybir.ActivationFunctionType
    Alu = mybir.AluOpType

    ng = bpc // GB
    nc = bacc.Bacc("TRN2", target_bir_lowering=False, debug=False, num_devices=N_CORES)

    # ---- DRAM I/O (per-core shapes) ----
    # x[b, parity*64+G, kc, i] = X[b, (2kc+parity)*64+G, i]
    x_d = nc.dram_tensor("x", [bpc, 128, 16, IN_DIM], bf16, kind="ExternalInput")
    # xt2[b, q, r] = X[b, r, q % 64]   (stacked twice for row-packing)
    xt2_d = nc.dram_tensor("xt2", [bpc, 128, IN_CAPS], bf16, kind="ExternalInput")
    # xs[grp, i, p'] = sum_G X[b, n*64+G, i] at p' = parity*64 + b*16 + kc
    xs_d = nc.dram_tensor("xs", [ng, IN_DIM, 128], bf16, kind="ExternalInput")
    # w2 = W stacked twice: [128 (dup ind), 2048]
    w2_d = nc.dram_tensor("w2", [128, NUM * DIM], bf16, kind="ExternalInput")
    # wt[d, g, i] = W[i, g*64+d]
    wt_d = nc.dram_tensor("wt", [IN_DIM, 32, IN_DIM], bf16, kind="ExternalInput")
    wsum_d = nc.dram_tensor("wsum", [IN_DIM, DIM], bf16, kind="ExternalInput")
    i128_d = nc.dram_tensor("i128", [128, 128], bf16, kind="ExternalInput")
    e2_d = nc.dram_tensor("e2", [128, 64], f32, kind="ExternalInput")
    e3_d = nc.dram_tensor("e3", [64, 128], f32, kind="ExternalInput")
    out_d = nc.dram_tensor("out", [bpc, NUM, DIM], f32, kind="ExternalOutput")

    with tile.TileContext(nc) as tc:
        with (
            tc.tile_pool(name="const", bufs=1) as cpool,
            tc.tile_pool(name="inp", bufs=2) as ipool,
            tc.tile_pool(name="work", bufs=2) as wpool,
            tc.tile_pool(name="big", bufs=3) as bigpool,
            tc.tile_pool(name="ps_wave", bufs=2, space="PSUM") as ps_wave,
            tc.tile_pool(name="ps_z", bufs=2, space="PSUM") as ps_z,
            tc.tile_pool(name="ps_db", bufs=2, space="PSUM") as ps_db,
            tc.tile_pool(name="ps_o", bufs=1, space="PSUM") as ps_o,
            tc.tile_pool(name="ps_sm", bufs=1, space="PSUM") as ps_sm,
        ):
            wsum_t = cpool.tile([IN_DIM, DIM], bf16, tag="wsum")
            nc.sync.dma_start(wsum_t[:], wsum_d[:])
            i128_t = cpool.tile([128, 128], bf16, tag="i128")
            nc.sync.dma_start(i128_t[:], i128_d[:])
            wt_t = cpool.tile([IN_DIM, 32, IN_DIM], bf16, tag="wt")
            nc.sync.dma_start(wt_t[:], wt_d[:])
            w2_t = cpool.tile([128, NUM * DIM], bf16, tag="w2")
            nc.scalar.dma_start(w2_t[:], w2_d[:])
            e2_t = cpool.tile([128, 64], f32, tag="e2")
            nc.sync.dma_start(e2_t[:], e2_d[:])
            e3_t = cpool.tile([64, 128], f32, tag="e3")
            nc.sync.dma_start(e3_t[:], e3_d[:])
            eps_t = cpool.tile([128, 1], f32, tag="eps")
            nc.vector.memset(eps_t[:], EPS)

            def squash(o_ps, want_f32=False):
                """psum [128,64] -> (f32 sbuf or None, bf16 sbuf) squashed."""
                o_sb = wpool.tile([128, DIM], f32, tag="osb")
                nc.vector.tensor_copy(o_sb[:], o_ps[:])
                o2 = wpool.tile([128, DIM], f32, tag="o2")
                s0 = wpool.tile([128, 1], f32, tag="s0")
                nc.scalar.activation(o2[:], o_ps[:], Act.Square, accum_out=s0[:])
                u = wpool.tile([128, 1], f32, tag="u")
                nc.scalar.activation(u[:], s0[:], Act.Sqrt, bias=eps_t[:])
                v = wpool.tile([128, 1], f32, tag="v")
                nc.vector.tensor_scalar_add(v[:], s0[:], 1.0 + EPS)
                rv = wpool.tile([128, 1], f32, tag="rv")
                nc.vector.reciprocal(rv[:], v[:])
                f = wpool.tile([128, 1], f32, tag="f")
                nc.vector.tensor_mul(f[:], u[:], rv[:])
                o_f32 = None
                if want_f32:
                    o_f32 = wpool.tile([128, DIM], f32, tag="osqf")
                    nc.vector.tensor_scalar_mul(o_f32[:], o_sb[:], f[:])
                o_bf = wpool.tile([128, DIM], bf16, tag="osqb")
                nc.vector.tensor_scalar_mul(o_bf[:], o_sb[:], f[:])
                return o_f32, o_bf

            def transpose_o(o_bf):
                """[128,64] bf16 -> oT sbuf [64,128] bf16."""
                t_ps = ps_wave.tile([IN_DIM, 128], bf16, tag="pw")
                nc.tensor.transpose(t_ps[:], o_bf[:], i128_t[:])
                oT = wpool.tile([IN_DIM, 128], bf16, tag="oT")
                nc.vector.tensor_copy(oT[:], t_ps[:])
                return oT

            def zstep(oT):
                """oT [64,128] -> z2 sbuf [128, 32 g, 128 p'] bf16 (dup halves)."""
                z2 = bigpool.tile([128, 32, 128], bf16, tag="z2")
                for gw in range(8):  # waves of 4 g
                    z_ps = ps_z.tile([128, 4, 128], f32, tag="z")
                    for j in range(4):
                        g = gw * 4 + j
                        nc.tensor.matmul(
                            z_ps[0:64, j, :], lhsT=wt_t[:, g, :], rhs=oT[:],
                            start=True, stop=True,
                        )
                        nc.tensor.matmul(
                            z_ps[64:128, j, :], lhsT=wt_t[:, g, :], rhs=oT[:],
                            start=True, stop=True,
                        )
                    if gw % 2 == 0:
                        nc.scalar.copy(z2[:, gw * 4 : gw * 4 + 4, :], z_ps[:])
                    else:
                        nc.vector.tensor_copy(z2[:, gw * 4 : gw * 4 + 4, :], z_ps[:])
                return z2

            def dbstep(z2, xt2_g, b_prev):
                """-> new b sbuf [128, 32 g, 64 slot] f32."""
                nb = bigpool.tile([128, 32, 64], f32, tag="b")
                for bw in range(GB):  # one wave per batch: 16 slots
                    db_ps = ps_db.tile([128, 32, 16], f32, tag="db")
                    b = bw
                    for kc in range(16):
                        slot = b * 16 + kc
                        for parity in range(2):
                            h = parity * 64
                            n = 2 * kc + parity
                            nc.tensor.matmul(
                                db_ps[h : h + 64, :, kc],
                                lhsT=xt2_g[b][h : h + 64, n * 64 : (n + 1) * 64],
                                rhs=z2[h : h + 64, :, h + slot],
                                start=True, stop=True,
                            )
                    dst = nb[:, :, b * 16 : (b + 1) * 16]
                    if b_prev is None:
                        nc.vector.tensor_copy(dst, db_ps[:])
                    else:
                        nc.vector.tensor_add(
                            dst, b_prev[:, :, b * 16 : (b + 1) * 16], db_ps[:]
                        )
                return nb

            def softmax(b_sb):
                """b [128, 64 slot, 32 g] f32 -> c bf16 same shape."""
                expb = bigpool.tile([128, 32, 64], f32, tag="expb")
                nc.scalar.activation(expb[:], b_sb[:], Act.Exp)
                # T[p, g, b] = sum_kc expb[p, g, b*16+kc]  (kc innermost, contiguous)
                T = wpool.tile([128, 32, GB], f32, tag="T")
                nc.vector.tensor_reduce(
                    T[:].rearrange("p g b -> p (g b)").unsqueeze(-1).squeeze(-1),
                    expb[:].rearrange("p g (b kc) -> p (g b) kc", kc=16),
                    mybir.AxisListType.X, Alu.add,
                )
                # fold partition halves with tiny f32 matmuls (no DMA latency):
                # S[G, col] = sum_p e2[p, G] * T[p, col] = T[G, col] + T[G+64, col]
                S_ps = ps_sm.tile([IN_DIM, 32 * GB], f32, tag="sm")
                nc.tensor.matmul(
                    S_ps[:], lhsT=e2_t[:], rhs=T[:].rearrange("p g b -> p (g b)"),
                    start=True, stop=True,
                )
                rs = wpool.tile([IN_DIM, 32, GB], f32, tag="rs")
                nc.vector.reciprocal(rs[:].rearrange("p g b -> p (g b)"), S_ps[:])
                rs2_ps = ps_sm.tile([128, 32 * GB], f32, tag="sm")
                nc.tensor.matmul(
                    rs2_ps[:], lhsT=e3_t[:], rhs=rs[:].rearrange("p g b -> p (g b)"),
                    start=True, stop=True,
                )
                c_sb = bigpool.tile([128, 32, 64], bf16, tag="c")
                nc.vector.tensor_mul(
                    c_sb[:].rearrange("p g (b kc) -> p (g b) kc", kc=16),
                    expb[:].rearrange("p g (b kc) -> p (g b) kc", kc=16),
                    rs2_ps[:, :, None].to_broadcast([128, 32 * GB, 16]),
                )
                return c_sb

            def pstep(c_sb, x_g):
                """c [128,32,64] bf16 + per-batch x -> p_all sbuf [128, 64 slot, 32 g]."""
                p_all = bigpool.tile([128, 64, 32], bf16, tag="pall")
                for bw in range(GB):
                    p_ps = ps_wave.tile([128, 16, 32], f32, tag="pw")
                    b = bw
                    for kc in range(16):
                        slot = b * 16 + kc
                        for parity in range(2):
                            h = parity * 64
                            nc.tensor.matmul(
                                p_ps[h : h + 64, kc, :],
                                lhsT=x_g[b][h : h + 64, kc, :],
                                rhs=c_sb[h : h + 64, :, slot],
                                start=True, stop=True,
                            )
                    if bw % 2 == 0:
                        nc.scalar.copy(p_all[:, b * 16 : (b + 1) * 16, :], p_ps[:])
                    else:
                        nc.vector.tensor_copy(p_all[:, b * 16 : (b + 1) * 16, :], p_ps[:])
                return p_all

            def mm2(p_all, o_ps):
                # MM2: per g, row-packed even/odd halves accumulate into o-all.
                for g in range(32):
                    nc.tensor.matmul(
                        o_ps[0:64, :],
                        lhsT=p_all[0:64, :, g],
                        rhs=w2_t[0:64, g * 64 : (g + 1) * 64],
                        start=(g == 0), stop=(g == 31),
                        skip_group_check=True,
                    )
                    nc.tensor.matmul(
                        o_ps[64:128, :],
                        lhsT=p_all[64:128, :, g],
                        rhs=w2_t[64:128, g * 64 : (g + 1) * 64],
                        start=(g == 0), stop=(g == 31),
                        skip_group_check=True,
                    )

            # ================= interleaved group emission =================
            st = [dict() for _ in range(ng)]

            def ph_load(g_):
                grp, s_ = g_, st[g_]
                xs_t = ipool.tile([IN_DIM, 128], bf16, tag="xs")
                nc.sync.dma_start(xs_t[:], xs_d[grp])
                s_["xs"] = xs_t
                qs = [nc.sync, nc.scalar]
                s_["x_g"], s_["xt2_g"] = [], []
                for j in range(GB):
                    b = grp * GB + j
                    x2t = ipool.tile([128, IN_CAPS], bf16, tag=f"xt2{j}")
                    qs[j % 2].dma_start(x2t[:], xt2_d[b])
                    s_["xt2_g"].append(x2t)
                for j in range(GB):
                    b = grp * GB + j
                    xt = ipool.tile([128, 16, IN_DIM], bf16, tag=f"x{j}")
                    qs[(j + 1) % 2].dma_start(xt[:], x_d[b])
                    s_["x_g"].append(xt)

            def ph_iter0(g_):
                s_ = st[g_]
                o_ps = ps_o.tile([128, DIM], f32, tag="o")
                nc.tensor.matmul(o_ps[:], lhsT=s_["xs"][:], rhs=wsum_t[:], start=True, stop=True)
                s_["o_f32"], s_["o_bf"] = squash(o_ps)
                s_["b"] = None
                s_["m2s_count"] = 0

            def ph_tz(g_):
                s_ = st[g_]
                oT = transpose_o(s_["o_bf"])
                s_["z2"] = zstep(oT)

            def ph_db(g_):
                s_ = st[g_]
                s_["b"] = dbstep(s_["z2"], s_["xt2_g"], s_["b"])

            def ph_smp(g_):
                s_ = st[g_]
                c_sb = softmax(s_["b"])
                s_["pall"] = pstep(c_sb, s_["x_g"])

            def ph_m2s(g_):
                s_ = st[g_]
                o_ps = ps_o.tile([128, DIM], f32, tag="o")
                mm2(s_["pall"], o_ps)
                s_["m2s_count"] += 1
                s_["o_f32"], s_["o_bf"] = squash(o_ps, want_f32=(s_["m2s_count"] == 2))

            def ph_out(g_):
                grp, s_ = g_, st[g_]
                for j in range(GB):
                    b = grp * GB + j
                    for parity in range(2):
                        nc.gpsimd.dma_start(
                            out_d[b].rearrange("(kc par) d -> par kc d", par=2)[parity],
                            s_["o_f32"][parity * 64 + j * 16 : parity * 64 + (j + 1) * 16, :],
                        )

            phases = [ph_load, ph_iter0, ph_tz, ph_db, ph_smp, ph_m2s,
                      ph_tz, ph_db, ph_smp, ph_m2s, ph_out]
            OFFSET = 1
            for k in range(len(phases) + OFFSET * (ng - 1)):
                for grp in range(ng):
                    kk = k - OFFSET * grp
                    if 0 <= kk < len(phases):
                        phases[kk](grp)

    nc.compile()
    return nc


def _get_nc():
    if "nc" not in _CACHE:
        _CACHE["nc"] = _build_nc()
    return _CACHE["nc"]


def _prep_host_small(inputs, kern):
    """Host-side input prep; inputs [Bn, 2048, 64] with Bn a multiple of GB."""
    import ml_dtypes

    bf = ml_dtypes.bfloat16
    Bn = inputs.shape[0]
    ng = Bn // GB
    X = np.ascontiguousarray(inputs, dtype=np.float32)
    W = np.ascontiguousarray(kern.reshape(IN_DIM, NUM * DIM), dtype=np.float32)

    # x[b, parity*64+G, kc, i] = X[b, (2kc+parity)*64+G, i]
    xr = X.reshape(Bn, 16, 2, 64, IN_DIM)          # [b, kc, parity, G, i]
    x_h = np.ascontiguousarray(xr.transpose(0, 2, 3, 1, 4).reshape(Bn, 128, 16, IN_DIM))
    xt = X.transpose(0, 2, 1)                      # [b, i, r]
    xt2_h = np.ascontiguousarray(np.concatenate([xt, xt], axis=1))  # [b, 128, 2048]
    # xs[grp, i, parity*64 + bj*16 + kc] = sum_G X[b, (2kc+parity)*64+G, i]
    xsum = X.reshape(Bn, 16, 2, 64, IN_DIM).sum(axis=3)  # [b, kc, parity, i]
    xs_h = np.zeros((ng, IN_DIM, 128), np.float32)
    for grp in range(ng):
        for j in range(GB):
            for parity in range(2):
                # [i, kc] block
                blk = xsum[grp * GB + j, :, parity, :].T
                xs_h[grp, :, parity * 64 + j * 16 : parity * 64 + (j + 1) * 16] = blk
    w2_h = np.concatenate([W, W], axis=0)          # [128, 2048]
    wt_h = np.ascontiguousarray(W.reshape(IN_DIM, 32, 64).transpose(2, 1, 0))
    wsum_h = np.ascontiguousarray(W.reshape(IN_DIM, 32, 64).sum(axis=1) / 32.0)
    i128_h = np.eye(128, dtype=np.float32)
    eye64 = np.eye(64, dtype=np.float32)
    e2_h = np.concatenate([eye64, eye64], axis=0)  # [128, 64]
    e3_h = np.ascontiguousarray(e2_h.T)            # [64, 128]
    return (
        x_h.astype(bf), xt2_h.astype(bf), xs_h.astype(bf),
        w2_h.astype(bf), wt_h.astype(bf), wsum_h.astype(bf), i128_h.astype(bf),
        e2_h, e3_h,
    )


def _make_in_maps(inputs, kern):
    x_h, xt2_h, xs_h, w2_h, wt_h, wsum_h, i128_h, e2_h, e3_h = _prep_host_small(
        np.asarray(inputs), np.asarray(kern)
    )
    in_maps = []
    for c in range(N_CORES):
        sl = slice(c * BPC, (c + 1) * BPC)
        gsl = slice(c * NG, (c + 1) * NG)
        in_maps.append(
            {
                "x": x_h[sl], "xt2": xt2_h[sl], "xs": xs_h[gsl],
                "w2": w2_h, "wt": wt_h, "wsum": wsum_h, "i128": i128_h,
                "e2": e2_h, "e3": e3_h,
            }
        )
    return in_maps


def kernel(inputs, kernel, num_capsule=NUM, dim_capsule=DIM, routings=3, **_):
    from concourse.bass_utils import run_bass_kernel_spmd

    assert int(num_capsule) == NUM and int(dim_capsule) == DIM and int(routings) == 3
    nc = _get_nc()
    in_maps = _make_in_maps(inputs, kernel)
    res = run_bass_kernel_spmd(nc, in_maps, core_ids=list(range(N_CORES)))
    out = np.concatenate([res.results[c]["out"] for c in range(N_CORES)], axis=0)
    return out.astype(np.float32)

